# revision 1
# baseline (speedup 1.0000x reference)
"""Causal self-attention (B=2, T=2048, C=1024, NH=16) on 8 TRN2 NeuronCores.

Sharding: core = (b, g): b in {0,1} batches, g in {0..3} head-groups of 4
heads (2 pairs of 2).  Each core computes qkv for its 4 heads from x[b],
runs causal attention, and produces a partial output projection in bf16.
The host sums the 4 partials per batch in fp32 and adds biases (bqkv's v
component is folded into bproj host-side: y += bv exactly).

v2 layout notes (per core):
  - xt (C, T) bf16: contraction dim C on partitions for qk projection.
  - q/k computed transposed: qT/kT [128 = 2 heads x 64d, pair, T] via
    W.T @ x; q weights pre-scaled by 1/8.  Bias added on DVE evict.
  - v computed in NATURAL layout directly: v[t, d] = x.T @ Wv per token
    tile (lhsT = xt tile, rhs = Wv), so no PE transposes are needed.
    v_sb[:, tt, h, 0:64] = v values; col 64 is a constant ones column so
    the AV matmul also yields softmax row sums l.
  - Attention per pair pr, query-block ib (512 queries), j-tile jt (128
    keys): S^T tiles [128 keys, 2 heads, 512 q] via two row-tiled K=64
    matmuls (tile_position (0,0)/(64,0) -> concurrent on HW); for
    diagonal tiles the causal mask is FOLDED INTO the accumulation as a
    third matmul (identity lhsT x precomputed -100 triangle tile;
    exp(-100) underflows to exactly 0), keeping GPSIMD off the critical
    chain; one exp on ACT covers both heads; AV accumulation per head
    (K=128, M=65 incl. ones column).
  - Normalize: 1/l per head pair broadcast with ONE K=2 matmul against a
    0/1 selector (rows select head halves), evicted to SBUF once, then
    two [64, 512] DVE muls produce yt (normalized y^T) per pair.
  - Proj per token tile: lhsT = yt tiles, accumulate 2 head-pairs into
    PSUM, DVE evict bf16, DMA out.  Proj for block ib is emitted after
    attention of block ib+1 so the PE never stalls on normalize.
  - Schedule: qk chunks and v tiles are emitted one phase AHEAD of the
    attention group that consumes them (their DVE evictions finish during
    the previous ACT-bound group); proj blocks trail one group behind
    their normalize.  Input DMA is ordered by first use with the small
    weights on the parallel gpsimd SWDGE path.
  - PSUM: tag "st" [128,2,512]x3 (S^T / qk acc / rbc / proj) = 6 banks,
    tag "small" [65,512]x2 (AV accum + v acc) = 2 banks.
"""

import numpy as np

import concourse.bass as bass
import concourse.mybir as mybir
import concourse.tile as tile
from concourse import bacc
from concourse.bass import ts, ds
from concourse.bass_utils import run_bass_kernel_spmd

B, T_FULL, C = 2, 2048, 1024
NH, HD = 16, 64
N_CORES = 8
HPC = 4  # heads per core
BF16 = mybir.dt.bfloat16
FP32 = mybir.dt.float32
AF = mybir.ActivationFunctionType
ALU = mybir.AluOpType


def build_program(T=T_FULL, repeat=1):
    NIB = T // 512   # query blocks
    NTT = T // 128   # token tiles
    NCT = C // 128   # contraction tiles
    nc = bacc.Bacc(None, target_bir_lowering=False)

    x_d = nc.dram_tensor("xt", [C, T], BF16, kind="ExternalInput")
    wqk_d = nc.dram_tensor("wqk", [C, 512], BF16, kind="ExternalInput")
    wv_d = nc.dram_tensor("wv", [C, 256], BF16, kind="ExternalInput")
    bqk_d = nc.dram_tensor("bqk", [512], FP32, kind="ExternalInput")
    wp_d = nc.dram_tensor("wproj", [256, C], BF16, kind="ExternalInput")
    out_d = nc.dram_tensor("out", [T, C], BF16, kind="ExternalOutput")

    with tile.TileContext(nc) as tc:
        with (
            tc.tile_pool(name="sb", bufs=1) as sb,
            tc.tile_pool(name="wk", bufs=1) as wk,
            tc.tile_pool(name="ps", bufs=1, space="PSUM") as ps,
        ):
            # ---- persistent SBUF (hoisted out of the repeat loop) ----
            xt_sb = sb.tile([128, NCT, T], BF16, name="xt_sb")
            wqk_sb = sb.tile([128, NCT, 512], BF16, name="wqk_sb")
            wv_sb = sb.tile([128, NCT, 256], BF16, name="wv_sb")
            wp_sb = sb.tile([128, 2, C], BF16, name="wp_sb")
            bias_sb = sb.tile([128, 4], FP32, name="bias_sb")
            sel_sb = sb.tile([64, 128], BF16, name="sel_sb")
            qT_sb = sb.tile([128, 2, T], BF16, name="qT_sb")
            kT_sb = sb.tile([128, 2, T], BF16, name="kT_sb")
            v_sb = sb.tile([128, NTT, HPC, 65], BF16, name="v_sb")
            yt_sb = sb.tile([128, 2, T], BF16, name="yt_sb")
            rl_sb = sb.tile([64, 2, T], BF16, name="rl_sb")

            # h2=0 lives at partition 0, h2=1 at partition 32 (engine writes
            # must start at a 32-aligned partition); filler rows are zeroed so
            # the K=33 selector matmul adds exact zeros for them.
            nc.vector.memset(sel_sb[:, :], 0.0)
            nc.vector.memset(sel_sb[0:1, 0:64], 1.0)
            nc.vector.memset(sel_sb[32:33, 64:128], 1.0)
            nc.vector.memset(rl_sb[0:64, :, :], 0.0)
            nc.vector.memset(v_sb[:, :, :, 64:65], 1.0)
            # identity (PE mask-add lhsT) and the causal -100 triangle tile:
            # within a diagonal 128x128 block, query col < key row p is masked.
            id_sb = sb.tile([128, 128], BF16, name="id_sb")
            mask_sb = sb.tile([128, 2, 128], BF16, name="mask_sb")
            nc.vector.memset(id_sb[:, :], 1.0)
            nc.gpsimd.affine_select(
                out=id_sb[:, :], in_=id_sb[:, :], compare_op=ALU.is_equal,
                fill=0.0, base=0, pattern=[[1, 128]], channel_multiplier=-1,
            )
            nc.vector.memset(mask_sb[:, :, :], 0.0)
            nc.gpsimd.affine_select(
                out=mask_sb[:, :, :], in_=mask_sb[:, :, :],
                compare_op=ALU.is_ge, fill=-100.0, base=0,
                pattern=[[0, 2], [1, 128]], channel_multiplier=-1,
            )

            for _rep in range(repeat):
                # ---- input DMA, in consumption order ----
                # DMA transfers are near-serial on the HWDGE path, so order
                # them by first use: wqk half, xt chunk-0 halves, wqk half,
                # then remaining xt chunks.  Small weights go via the gpsimd
                # SWDGE path, which runs in parallel with HWDGE transfers.
                nc.gpsimd.dma_start(
                    out=bias_sb[:, :],
                    in_=bqk_d.ap().rearrange("(a p) -> p a", p=128),
                )
                nc.gpsimd.dma_start(
                    out=wv_sb[:, :, :],
                    in_=wv_d.ap().rearrange("(c p) f -> p c f", p=128),
                )
                def dma_wqk(h):
                    nc.sync.dma_start(
                        out=wqk_sb[:, ds(4 * h, 4), :],
                        in_=wqk_d[ds(512 * h, 512), :].rearrange(
                            "(c p) f -> p c f", p=128
                        ),
                    )
                def dma_xt(ci0, nci, tp):
                    nc.sync.dma_start(
                        out=xt_sb[:, ds(ci0, nci), ts(tp, 512)],
                        in_=x_d[ds(128 * ci0, 128 * nci), ts(tp, 512)].rearrange(
                            "(c p) f -> p c f", p=128
                        ),
                    )
                dma_wqk(0)
                dma_xt(0, 4, 0)
                dma_wqk(1)
                dma_xt(4, 4, 0)
                nc.gpsimd.dma_start(
                    out=wp_sb[:, :, :],
                    in_=wp_d.ap().rearrange("(a p) f -> p a f", p=128),
                )
                for tp in range(1, NIB):
                    dma_xt(0, 8, tp)

                # ---- building blocks ----
                def qk_half(pr, tp, which):
                    """q or k projection for pair pr, 512-token chunk tp."""
                    ft = pr if which == 0 else 2 + pr
                    dest = qT_sb if which == 0 else kT_sb
                    acc = ps.tile([128, 2, 512], FP32, name="qkacc",
                                  tag="st", bufs=3)
                    for ci in range(NCT):
                        nc.tensor.matmul(
                            acc[:, 0, :],
                            wqk_sb[:, ci, ts(ft, 128)],
                            xt_sb[:, ci, ts(tp, 512)],
                            start=(ci == 0),
                            stop=(ci == NCT - 1),
                        )
                    nc.vector.tensor_scalar_add(
                        dest[:, pr, ts(tp, 512)],
                        acc[:, 0, :],
                        bias_sb[:, ft : ft + 1],
                    )

                def qk_chunk(pr, tp):
                    qk_half(pr, tp, 0)
                    qk_half(pr, tp, 1)

                def v_tile(tt):
                    """v in natural layout for token tile tt (4 heads)."""
                    vacc = ps.tile([128, 256], FP32, name="vacc",
                                   tag="small", bufs=2)
                    for ci in range(NCT):
                        nc.tensor.matmul(
                            vacc[:, :],
                            xt_sb[:, ci, ts(tt, 128)],
                            wv_sb[:, ci, :],
                            start=(ci == 0),
                            stop=(ci == NCT - 1),
                        )
                    nc.vector.tensor_copy(
                        v_sb[:, tt, :, 0:64],
                        vacc.rearrange("p (h d) -> p h d", h=HPC),
                    )

                def attn_group(pr, ib, fillers=()):
                    """QK^T -> exp(+mask) -> AV for 512 queries, both heads.

                    fillers: small PE work units injected between j-tiles so
                    the in-order PE fills its slack during the ACT-bound
                    stretch instead of running them serially afterwards."""
                    njt = 4 * (ib + 1)
                    fillers = list(fillers)
                    gap = max(2, njt // (len(fillers) + 1)) if fillers else njt
                    av = None

                    def ensure_av():
                        nonlocal av
                        if av is None:
                            av = [
                                ps.tile([65, 512], FP32, name=f"av{h2}",
                                        tag="small", bufs=2)
                                for h2 in range(2)
                            ]
                    for jt in range(njt):
                        a = jt - 4 * ib
                        off = 128 * a if a >= 0 else 0
                        w = 512 - off
                        st = ps.tile([128, 2, 512], FP32, name="st",
                                     tag="st", bufs=3)
                        pt = wk.tile([128, 2, 512], BF16, name="pt",
                                     tag="pt", bufs=6)
                        diag = a >= 0
                        for h2 in range(2):
                            nc.tensor.matmul(
                                st[:, h2, ds(off, w)],
                                kT_sb[ds(64 * h2, 64), pr, ts(jt, 128)],
                                qT_sb[ds(64 * h2, 64), pr,
                                      ds(512 * ib + off, w)],
                                start=True,
                                stop=not diag,
                                skip_group_check=diag,
                            )
                        if diag:
                            # causal mask folded into the QK accumulation:
                            # -100 on masked elements -> exp underflows to 0
                            nc.tensor.matmul(
                                st[:, :, ds(off, 128)],
                                id_sb[:, :],
                                mask_sb[:, :, :],
                                start=False,
                                stop=True,
                                skip_group_check=True,
                            )
                        nc.scalar.activation(
                            pt[:, :, ds(off, w)], st[:, :, ds(off, w)], AF.Exp
                        )
                        ensure_av()
                        for h2 in range(2):
                            nc.tensor.matmul(
                                av[h2][:, ds(off, w)],
                                v_sb[:, jt, 2 * pr + h2, :],
                                pt[:, h2, ds(off, w)],
                                start=(jt == 0),
                                stop=(jt == njt - 1),
                            )
                        if fillers and jt % gap == gap - 1 and jt < njt - 1:
                            fillers.pop(0)()
                    for f in fillers:
                        f()
                    return av

                def normalize(pr, ib, av):
                    """yt[:, pr, ib-block] = av / l (broadcast 1/l, 2 heads)."""
                    blk = ts(ib, 512)
                    # bf16 1/l: a 0.4%-rel multiplicative error on the
                    # normalization, well inside the output tolerance; bf16
                    # keeps the broadcast matmul at 1 cycle/row.
                    with nc.allow_low_precision(reason="bf16 1/l broadcast"):
                        for h2 in range(2):
                            nc.vector.reciprocal(
                                rl_sb[32 * h2 : 32 * h2 + 1, pr, blk],
                                av[h2][64:65, :],
                            )
                    rbc = ps.tile([128, 2, 512], FP32, name="rbc",
                                  tag="st", bufs=3)
                    nc.tensor.matmul(
                        rbc[:, 0, :],
                        sel_sb[0:33, :],
                        rl_sb[0:33, pr, blk],
                        start=True,
                        stop=True,
                    )
                    rbc_sb = wk.tile([128, 512], BF16, name="rbc_sb",
                                     tag="rbcsb", bufs=2)
                    nc.vector.tensor_copy(rbc_sb[:, :], rbc[:, 0, :])
                    for h2 in range(2):
                        nc.vector.tensor_mul(
                            yt_sb[ds(64 * h2, 64), pr, blk],
                            av[h2][0:64, :],
                            rbc_sb[ds(64 * h2, 64), :],
                        )

                def proj_tile(tt, last=False):
                    """output projection + DMA for one token tile."""
                    if True:
                        pp = ps.tile([128, 2, 512], FP32, name="pp",
                                     tag="st", bufs=3)
                        for hd in range(2):
                            for oc in range(2):
                                nc.tensor.matmul(
                                    pp[:, oc, :],
                                    yt_sb[:, hd, ts(tt, 128)],
                                    wp_sb[:, hd, ts(oc, 512)],
                                    start=(hd == 0),
                                    stop=(hd == 1),
                                )
                        outst = wk.tile([128, 1024], BF16, name="outst",
                                        tag="outst", bufs=2)
                        # final block: split evictions DVE/ACT (shorter tail)
                        if last and tt % 2 == 1:
                            nc.scalar.activation(
                                outst.rearrange("p (a b) -> p a b", a=2),
                                pp[:, :, :],
                                AF.Copy,
                            )
                        else:
                            nc.vector.tensor_copy(
                                outst.rearrange("p (a b) -> p a b", a=2), pp[:, :, :]
                            )
                        nc.sync.dma_start(out=out_d[ts(tt, 128), :], in_=outst[:, :])

                def proj_block(ib, last=False):
                    for tt in range(4 * ib, 4 * ib + 4):
                        proj_tile(tt, last)

                # ---- schedule ----
                # qk halves and proj tiles for LATER phases are injected as
                # fillers inside the ACT-bound attention groups (one phase
                # ahead of their consumer, one behind their producer), so
                # their DVE evictions and PE time hide in the groups' slack.
                qk_chunk(0, 0)
                for ib in range(NIB):
                    for tt in range(4 * ib, 4 * ib + 4):
                        v_tile(tt)
                    nxt = (0, ib + 1) if ib + 1 < NIB else (1, 0)
                    fillers = [
                        (lambda pr_tp=nxt: qk_half(pr_tp[0], pr_tp[1], 0)),
                        (lambda pr_tp=nxt: qk_half(pr_tp[0], pr_tp[1], 1)),
                    ]
                    av = attn_group(0, ib, fillers)
                    normalize(0, ib, av)
                for ib in range(NIB):
                    fillers = []
                    if ib + 1 < NIB:
                        fillers += [
                            (lambda w=w, i=ib + 1: qk_half(1, i, w))
                            for w in range(2)
                        ]
                    if ib > 0:
                        fillers += [
                            (lambda t=tt: proj_tile(t))
                            for tt in range(4 * (ib - 1), 4 * (ib - 1) + 4)
                        ]
                    av = attn_group(1, ib, fillers)
                    normalize(1, ib, av)
                proj_block(NIB - 1, last=True)

    nc.compile()
    return nc


def _prep_inputs(x, Wqkv, bqkv, Wproj, T=T_FULL):
    """Build the 8 per-core input maps (host-side shard/cast/transpose)."""
    import ml_dtypes

    bf16 = ml_dtypes.bfloat16
    x = np.asarray(x, dtype=np.float32)
    Wqkv = np.asarray(Wqkv, dtype=np.float32)
    bqkv = np.asarray(bqkv, dtype=np.float32)
    Wproj = np.asarray(Wproj, dtype=np.float32)

    in_maps = []
    for b in range(B):
        xt = np.ascontiguousarray(x[b, :T].T).astype(bf16)  # (C, T)
        for g in range(N_CORES // B):
            heads = [4 * g + h for h in range(HPC)]
            wq = np.concatenate(
                [Wqkv[:, hh * HD : (hh + 1) * HD] for hh in heads], axis=1
            ) * 0.125
            wk_ = np.concatenate(
                [Wqkv[:, C + hh * HD : C + (hh + 1) * HD] for hh in heads],
                axis=1,
            )
            wqk = np.concatenate([wq, wk_], axis=1).astype(bf16)  # (C, 512)
            wv = np.concatenate(
                [Wqkv[:, 2 * C + hh * HD : 2 * C + (hh + 1) * HD] for hh in heads],
                axis=1,
            ).astype(bf16)  # (C, 256)
            bq = np.concatenate(
                [bqkv[hh * HD : (hh + 1) * HD] for hh in heads]
            ) * 0.125
            bk = np.concatenate(
                [bqkv[C + hh * HD : C + (hh + 1) * HD] for hh in heads]
            )
            bqk = np.concatenate([bq, bk]).astype(np.float32)  # (512,)
            wp = np.concatenate(
                [Wproj[hh * HD : (hh + 1) * HD, :] for hh in heads], axis=0
            ).astype(bf16)  # (256, C)
            in_maps.append({"xt": xt, "wqk": wqk, "wv": wv, "bqk": bqk, "wproj": wp})
    return in_maps


_PROGRAM_CACHE = {}


def get_program(T=T_FULL, repeat=1):
    key = (T, repeat)
    if key not in _PROGRAM_CACHE:
        _PROGRAM_CACHE[key] = build_program(T, repeat)
    return _PROGRAM_CACHE[key]


def kernel(x, Wqkv, bqkv, Wproj, bproj):
    x = np.asarray(x)
    in_dtype = x.dtype
    nc = get_program(T_FULL)
    in_maps = _prep_inputs(x, Wqkv, bqkv, Wproj)
    res = run_bass_kernel_spmd(nc, in_maps, list(range(N_CORES))).results
    gpb = N_CORES // B
    bqkv = np.asarray(bqkv, dtype=np.float32)
    bproj = np.asarray(bproj, dtype=np.float32)
    # fold the v bias exactly: y = attn(x) + bv  =>  out += bv @ Wproj
    bproj_eff = bproj + np.asarray(bqkv[2 * C :], dtype=np.float32) @ np.asarray(
        Wproj, dtype=np.float32
    )
    out = np.stack(
        [
            sum(res[b * gpb + g]["out"].astype(np.float32) for g in range(gpb))
            + bproj_eff
            for b in range(B)
        ]
    )
    return out.astype(in_dtype)

